# revision 46
# baseline (speedup 1.0000x reference)
"""Trainium2 Bass kernel for the GNN-RNN diagnoser.

Model (per reference): x [8192,1,5,128]; 128 sequential timesteps; each step
runs 5 per-node MLPs (130->256->256->64, relu/relu/tanh) where node inputs are
[x of 2 parents ; states of 2 parents]; then head 325->256 relu, BatchNorm
(training mode, full-batch stats), 256->7, softmax.

Strategy (v5):
  - Data-parallel: batch 8192 -> 8 cores x 1024; feature-major layout
    [features, batch] on device.
  - States kept as 5 "parent pair" SBUF tensors P[m] = [s_p0 ; s_p1] (128 x B)
    in bf16, ping-ponged across steps, so layer-1 is one K=128 matmul per
    m-chunk plus a small K=10 x-injection matmul into the same PSUM bank.
  - L1 emission: 4 full-row state matmuls first (start=True), then the 4
    x-injection matmuls back-to-back on the 4 PE row groups (concurrent).
  - Layer-2 in fp8-e4m3 DoubleRow: h1 written as [128,2(kc),B] fp8, W2
    packed [128,2,128]; one matmul contracts K=256 at 2 MACs/cycle.
    (Simulated end-to-end error 6e-3 vs the 2e-2 budget.)
  - The whole per-node chain is ncs-major (512-batch half-slices flow
    L1->relu1->L2->relu2->L3->tanh independently) to shorten the recurrence
    critical path.
  - Layer-3 column-paired; pair (2,3) swaps rows (s3 on 0:64, s2 on 64:128)
    so tanh writes P0/P3 slots partition-aligned: the recurrence-critical
    state hand-offs never ride a DMA.  P4/P3/P1 secondary slots are filled
    by GpSimd-triggered SBUF DMAs on slack paths.
  - PSUM-draining activations greedily balanced between Vector and Scalar;
    tanh and state hand-offs are scheduler-prioritized.
  - BatchNorm/Wo2/softmax tail on host (needs cross-core batch stats).
"""

import numpy as np
import ml_dtypes
from contextlib import ExitStack

import concourse.bass as bass
import concourse.bacc as bacc
import concourse.tile as tile
from concourse import mybir
from concourse.bass_utils import run_bass_kernel_spmd

F32 = mybir.dt.float32
BF16 = mybir.dt.bfloat16
FP8 = mybir.dt.float8e4
AF = mybir.ActivationFunctionType
ALU = mybir.AluOpType
DR = mybir.MatmulPerfMode.DoubleRow

NCORES = 8
B = 8192
BC = B // NCORES          # 1024 batch per core
T = 128
TC = 8                    # timesteps per xg ring-buffer chunk
NODE = 5
FML = 64
F0 = 256
NCS = 2                   # batch N-chunks of 512 (PSUM bank limit)
NW = BC // NCS            # 512
XROWS = 106               # xg replicated at row groups 0,32,64,96 (10 rows each)

PARENTS = [(3, 4), (0, 4), (0, 1), (1, 2), (2, 3)]
BN_EPS = 1e-5


def _build_nc():
    nc = bacc.Bacc("TRN2", target_bir_lowering=False, debug=False,
                   num_devices=NCORES)

    # ---- DRAM I/O -------------------------------------------------------
    xg_d = nc.dram_tensor("xg", [XROWS, T // TC, TC * BC], BF16, kind="ExternalInput")
    xt_d = nc.dram_tensor("xt", [NODE, BC], F32, kind="ExternalInput")
    w1s_d = nc.dram_tensor("w1s", [128, NODE * F0], BF16, kind="ExternalInput")
    w1x_d = nc.dram_tensor("w1x", [XROWS, NODE * F0], BF16, kind="ExternalInput")
    w2s_d = nc.dram_tensor("w2s", [128, NODE * 2, 2, 128], FP8, kind="ExternalInput")
    w3s_d = nc.dram_tensor("w3s", [128, NODE * 2 * FML], BF16, kind="ExternalInput")
    b1_d = nc.dram_tensor("b1p", [128, NODE * 2], F32, kind="ExternalInput")
    b2_d = nc.dram_tensor("b2p", [128, NODE * 2], F32, kind="ExternalInput")
    b3_d = nc.dram_tensor("b3p", [128, 3], F32, kind="ExternalInput")
    wo1a_d = nc.dram_tensor("wo1a", [NODE, F0], F32, kind="ExternalInput")
    wo1b_d = nc.dram_tensor("wo1b", [128, F0], F32, kind="ExternalInput")
    wo1c_d = nc.dram_tensor("wo1c", [128, F0], F32, kind="ExternalInput")
    wo1d_d = nc.dram_tensor("wo1d", [64, F0], F32, kind="ExternalInput")
    bo1_d = nc.dram_tensor("bo1p", [128, 2], F32, kind="ExternalInput")
    h0_d = nc.dram_tensor("h0", [128, BC], F32, kind="ExternalOutput")
    h1_d = nc.dram_tensor("h1", [128, BC], F32, kind="ExternalOutput")

    with tile.TileContext(nc) as tc, ExitStack() as ctx:
        const = ctx.enter_context(tc.tile_pool(name="const", bufs=1))
        spool = ctx.enter_context(tc.tile_pool(name="state", bufs=1))
        apool = ctx.enter_context(tc.tile_pool(name="act", bufs=3))
        xpool = ctx.enter_context(tc.tile_pool(name="xgr", bufs=4))
        hpool = ctx.enter_context(tc.tile_pool(name="head", bufs=1))
        ps1 = ctx.enter_context(tc.tile_pool(name="ps1", bufs=4, space="PSUM"))
        ps2 = ctx.enter_context(tc.tile_pool(name="ps2", bufs=3, space="PSUM"))
        ps3 = ctx.enter_context(tc.tile_pool(name="ps3", bufs=1, space="PSUM"))

        # Greedy build-time load balancing of PSUM-draining activations
        # between the Vector and Scalar engines (tanh is Scalar-only).
        ebusy = {"v": 0.0, "s": 0.0}

        def relu_ts(out, in_, bias_col, cost_v=700.0, cost_s=640.0,
                    force=None):
            """relu(x + b): tensor_scalar on DVE or activation on ScalarE.

            Returns the engine used; force="v"/"s" overrides the greedy
            choice (used to pin the two halves of a relu1 pair to opposite
            engines so they always run in parallel)."""
            on_v = (ebusy["v"] + cost_v <= ebusy["s"] + cost_s
                    if force is None else force == "v")
            if on_v:
                ebusy["v"] += cost_v
                nc.vector.tensor_scalar(out, in_, bias_col, 0.0,
                                        ALU.add, ALU.max)
                return "v"
            ebusy["s"] += cost_s
            nc.scalar.activation(out, in_, AF.Relu, bias=bias_col)
            return "s"

        def load(pool, dram, shape, dtype, tag, eng=None):
            t = pool.tile(shape, dtype, tag=tag, name=tag)
            (eng or nc.sync).dma_start(out=t[:], in_=dram[:])
            return t

        # big constants split across the sync- and scalar-triggered DMA
        # queues so the startup transfers overlap
        w1s = load(const, w1s_d, [128, NODE * F0], BF16, "w1s")
        w1x = load(const, w1x_d, [XROWS, NODE * F0], BF16, "w1x",
                   eng=nc.scalar)
        w2s = load(const, w2s_d, [128, NODE * 2, 2, 128], FP8, "w2s",
                   eng=nc.scalar)
        w3s = load(const, w3s_d, [128, NODE * 2 * FML], BF16, "w3s")
        b1 = load(const, b1_d, [128, NODE * 2], F32, "b1", eng=nc.scalar)
        b2 = load(const, b2_d, [128, NODE * 2], F32, "b2")
        b3 = load(const, b3_d, [128, 3], F32, "b3")

        # state pair buffers, ping-pong
        P = [[spool.tile([128, BC], BF16, tag=f"P{ph}_{m}", name=f"P{ph}_{m}")
              for m in range(NODE)] for ph in range(2)]
        for m in range(NODE):
            nc.vector.memset(P[0][m][:], 0.0)

        xg_ring = {}

        def layers12(n, cur, xr, tl, hot_l2=False, l3fn=None):
            """L1 + L2 for node n; returns the two h2 M-chunk tiles.

            ncs-major: the two 512-batch half-slices flow through
            L1 -> relu1 -> L2(DoubleRow) -> relu2 independently."""
            pts = [ps1.tile([128, NW], F32, tag="ps1", name="ps1t")
                   for _ in range(4)]          # index 2*ncs + mc
            # per half-slice: 2 state matmuls then the 2 x-injection closers
            # (row-tiled, concurrent) so relu1(ncs0) is ready two matmul
            # slots earlier and the ncs0 stream truly leads.
            for ncs in range(NCS):
                for mc in range(2):
                    nc.tensor.matmul(
                        pts[2 * ncs + mc][:],
                        lhsT=w1s[:, n * F0 + mc * 128: n * F0 + (mc + 1) * 128],
                        rhs=cur[n][:, ncs * NW:(ncs + 1) * NW],
                        start=True, stop=False)
                for mc in range(2):
                    g = 2 * ncs + mc
                    rb = 32 * g
                    nc.tensor.matmul(
                        pts[g][:],
                        lhsT=w1x[rb:rb + 10, n * F0 + mc * 128:n * F0 + (mc + 1) * 128],
                        rhs=xr[rb:rb + 10, tl, ncs * NW:(ncs + 1) * NW],
                        start=False, stop=True, tile_position=(rb, 0))
            h1dr = apool.tile([128, 2, BC], FP8, tag="h1dr", name="h1dr")
            h2t = [apool.tile([128, BC], BF16, tag=f"h2_{mc}", name=f"h2_{mc}",
                              bufs=5)
                   for mc in range(2)]
            for ncs in range(NCS):
                sl = slice(ncs * NW, (ncs + 1) * NW)
                with tc.high_priority(offset=90):
                    e0 = relu_ts(h1dr[:, 0, sl], pts[2 * ncs + 0][:],
                                 b1[:, n * 2 + 0: n * 2 + 1])
                    relu_ts(h1dr[:, 1, sl], pts[2 * ncs + 1][:],
                            b1[:, n * 2 + 1: n * 2 + 2],
                            force=("s" if e0 == "v" else "v"))
                pt2 = [ps2.tile([128, NW], F32, tag="ps2", name="ps2t")
                       for _ in range(2)]
                # one DoubleRow matmul contracts all K=256 per bank
                for mc in range(2):
                    nc.tensor.matmul(
                        pt2[mc][:],
                        lhsT=w2s[:, n * 2 + mc, :, :],
                        rhs=h1dr[:, :, sl],
                        start=True, stop=True, perf_mode=DR)
                if hot_l2:
                    with tc.high_priority(offset=90):
                        for mc in range(2):
                            relu_ts(h2t[mc][:, sl], pt2[mc][:],
                                    b2[:, n * 2 + mc: n * 2 + mc + 1])
                else:
                    for mc in range(2):
                        relu_ts(h2t[mc][:, sl], pt2[mc][:],
                                b2[:, n * 2 + mc: n * 2 + mc + 1])
                if l3fn is not None:
                    l3fn(h2t, ncs)
            return h2t

        def l3_pair01_ncs(h2A, h2B, nxt, ncs):
            """One ncs half-slice of pair (0,1): matmuls + tanh into P2."""
            dest = nxt[2]
            sl = slice(ncs * NW, (ncs + 1) * NW)
            pt = ps3.tile([128, NW], F32, tag="ps3", name="ps3t")
            for kc in range(2):
                nc.tensor.matmul(
                    pt[0:64, :],
                    lhsT=w3s[:, (0 * 2 + kc) * FML:(0 * 2 + kc + 1) * FML],
                    rhs=h2A[kc][:, sl],
                    start=(kc == 0), stop=(kc == 1), tile_position=(0, 0))
                nc.tensor.matmul(
                    pt[64:128, :],
                    lhsT=w3s[:, (1 * 2 + kc) * FML:(1 * 2 + kc + 1) * FML],
                    rhs=h2B[kc][:, sl],
                    start=(kc == 0), stop=(kc == 1), tile_position=(0, 64))
            ebusy["s"] += 613.0
            with tc.high_priority(offset=130):
                nc.scalar.activation(dest[:, sl], pt[:], AF.Tanh,
                                     bias=b3[:, 0:1])

        def l3_pair01_tail(nxt):
            dest = nxt[2]
            with tc.high_priority(offset=130):
                ebusy["v"] += 420.0
                nc.vector.tensor_copy(out=nxt[1][0:64, :], in_=dest[0:64, :])
                nc.gpsimd.dma_start(out=nxt[3][0:64, :], in_=dest[64:128, :])

        def l3_pair01(h2A, h2B, nxt):
            """Column-paired DoubleRow layer 3 for nodes (0,1): psum rows
            = [s0; s1].

            tanh writes P2 (= [s0;s1]) directly; s0 -> P1[0:64] same-row
            copy (GpSimd); s1 -> P3[0:64] is cross-partition, DMA'd on a
            slack path (node 3 runs last next step)."""
            dest = nxt[2]
            for ncs in range(NCS):
                sl = slice(ncs * NW, (ncs + 1) * NW)
                pt = ps3.tile([128, NW], F32, tag="ps3", name="ps3t")
                for kc in range(2):
                    nc.tensor.matmul(
                        pt[0:64, :],
                        lhsT=w3s[:, (0 * 2 + kc) * FML:(0 * 2 + kc + 1) * FML],
                        rhs=h2A[kc][:, sl],
                        start=(kc == 0), stop=(kc == 1), tile_position=(0, 0))
                    nc.tensor.matmul(
                        pt[64:128, :],
                        lhsT=w3s[:, (1 * 2 + kc) * FML:(1 * 2 + kc + 1) * FML],
                        rhs=h2B[kc][:, sl],
                        start=(kc == 0), stop=(kc == 1), tile_position=(0, 64))
                ebusy["s"] += 613.0
                with tc.high_priority(offset=130):
                    nc.scalar.activation(dest[:, sl], pt[:], AF.Tanh,
                                         bias=b3[:, 0:1])
            with tc.high_priority(offset=130):
                ebusy["v"] += 420.0
                nc.vector.tensor_copy(out=nxt[1][0:64, :], in_=dest[0:64, :])
                nc.gpsimd.dma_start(out=nxt[3][0:64, :], in_=dest[64:128, :])

        def l3_pair23_ncs(h2_3, h2_2, nxt, ncs):
            """One ncs half-slice of pair (3,2): matmuls + tanh into P4."""
            dest = nxt[4]
            sl = slice(ncs * NW, (ncs + 1) * NW)
            pt = ps3.tile([128, NW], F32, tag="ps3", name="ps3t")
            for kc in range(2):
                nc.tensor.matmul(
                    pt[0:64, :],
                    lhsT=w3s[:, (3 * 2 + kc) * FML:(3 * 2 + kc + 1) * FML],
                    rhs=h2_3[kc][:, sl],
                    start=(kc == 0), stop=(kc == 1), tile_position=(0, 0))
                nc.tensor.matmul(
                    pt[64:128, :],
                    lhsT=w3s[:, (2 * 2 + kc) * FML:(2 * 2 + kc + 1) * FML],
                    rhs=h2_2[kc][:, sl],
                    start=(kc == 0), stop=(kc == 1), tile_position=(0, 64))
            ebusy["s"] += 613.0
            with tc.high_priority(offset=130):
                nc.scalar.activation(dest[:, sl], pt[:], AF.Tanh,
                                     bias=b3[:, 1:2])

        def l3_pair23_tail(nxt):
            dest = nxt[4]
            with tc.high_priority(offset=130):
                ebusy["v"] += 840.0
                nc.vector.tensor_copy(out=nxt[0][0:64, :], in_=dest[0:64, :])
                nc.vector.tensor_copy(out=nxt[3][64:128, :], in_=dest[64:128, :])

        def l3_pair23(h2_3, h2_2, nxt):
            """Column-paired DoubleRow layer 3 for nodes (3,2): psum rows
            = [s3; s2].

            Node 4's W1 state-weight halves are swapped host-side, so P4
            stores [s3; s2] and tanh writes it DIRECTLY.  The other
            consumers are partition-aligned copies: s3 -> P0[0:64] (node 0,
            Vector, urgent), s2 -> P3[64:128] (node 3, GpSimd, slack)."""
            dest = nxt[4]
            for ncs in range(NCS):
                sl = slice(ncs * NW, (ncs + 1) * NW)
                pt = ps3.tile([128, NW], F32, tag="ps3", name="ps3t")
                for kc in range(2):
                    nc.tensor.matmul(
                        pt[0:64, :],
                        lhsT=w3s[:, (3 * 2 + kc) * FML:(3 * 2 + kc + 1) * FML],
                        rhs=h2_3[kc][:, sl],
                        start=(kc == 0), stop=(kc == 1), tile_position=(0, 0))
                    nc.tensor.matmul(
                        pt[64:128, :],
                        lhsT=w3s[:, (2 * 2 + kc) * FML:(2 * 2 + kc + 1) * FML],
                        rhs=h2_2[kc][:, sl],
                        start=(kc == 0), stop=(kc == 1), tile_position=(0, 64))
                ebusy["s"] += 613.0
                with tc.high_priority(offset=130):
                    nc.scalar.activation(dest[:, sl], pt[:], AF.Tanh,
                                         bias=b3[:, 1:2])
            with tc.high_priority(offset=130):
                ebusy["v"] += 840.0
                nc.vector.tensor_copy(out=nxt[0][0:64, :], in_=dest[0:64, :])
                nc.vector.tensor_copy(out=nxt[3][64:128, :], in_=dest[64:128, :])

        def l3_solo4(h2t, nxt):
            """Layer 3 for node 4: both its slots sit at row 64."""
            for ncs in range(NCS):
                sl = slice(ncs * NW, (ncs + 1) * NW)
                pt = ps3.tile([128, NW], F32, tag="ps3", name="ps3t")
                for kc in range(2):
                    nc.tensor.matmul(
                        pt[64:128, :],
                        lhsT=w3s[:, (4 * 2 + kc) * FML:(4 * 2 + kc + 1) * FML],
                        rhs=h2t[kc][:, sl],
                        start=(kc == 0), stop=(kc == 1), tile_position=(0, 64))
                ebusy["s"] += 613.0
                with tc.high_priority(offset=130):
                    nc.scalar.activation(nxt[0][64:128, sl], pt[64:128, :],
                                         AF.Tanh, bias=b3[64:128, 2:3])
            with tc.high_priority(offset=130):
                ebusy["v"] += 420.0
                nc.vector.tensor_copy(out=nxt[1][64:128, :], in_=nxt[0][64:128, :])

        def fetch_chunk(c):
            if c in xg_ring or c >= T // TC:
                return
            # issue on GpSimd: its instruction stream is otherwise empty, so
            # the trigger fires as soon as the ring slot frees (true prefetch)
            xr = xpool.tile([XROWS, TC, BC], BF16, tag="xgr", name="xgr")
            q = (TC // 4) * BC
            for s4 in range(4):
                nc.gpsimd.dma_start(
                    out=xr[:, s4 * (TC // 4):(s4 + 1) * (TC // 4), :],
                    in_=xg_d[:, c, s4 * q:(s4 + 1) * q])
            xg_ring[c] = xr

        def step(t_abs):
            cur = P[t_abs % 2]
            nxt = P[(t_abs + 1) % 2]
            c, tl = divmod(t_abs, TC)
            fetch_chunk(c)
            if tl == 2:
                fetch_chunk(c + 1)   # prefetch next chunk early in this one
            xr = xg_ring[c]
            # Node order [2,0,1,4,3]: consumers of the earliest-completed state
            # pairs run first next step, so cross-step producer->consumer
            # latency (tanh + copy/DMA chains) is hidden.
            h2_2 = layers12(2, cur, xr, tl, hot_l2=True)
            h2_1 = layers12(1, cur, xr, tl, hot_l2=True)
            h2_0 = layers12(0, cur, xr, tl, hot_l2=True,
                            l3fn=lambda h2t, ncs:
                                l3_pair01_ncs(h2t, h2_1, nxt, ncs))
            l3_pair01_tail(nxt)
            h2_4 = layers12(4, cur, xr, tl, hot_l2=True)
            l3_solo4(h2_4, nxt)
            h2_3 = layers12(3, cur, xr, tl, hot_l2=True,
                            l3fn=lambda h2t, ncs:
                                l3_pair23_ncs(h2t, h2_2, nxt, ncs))
            l3_pair23_tail(nxt)

        for t_abs in range(T):
            step(t_abs)

        # head-only constants: loaded during the loop, not at startup
        xt = load(const, xt_d, [NODE, BC], F32, "xt")
        wo1a = load(const, wo1a_d, [NODE, F0], F32, "wo1a")
        wo1b = load(const, wo1b_d, [128, F0], F32, "wo1b")
        wo1c = load(const, wo1c_d, [128, F0], F32, "wo1c")
        wo1d = load(const, wo1d_d, [64, F0], F32, "wo1d")
        bo1 = load(const, bo1_d, [128, 2], F32, "bo1")

        # ---- head: feat = [x_T(5); s0..s4(320)] -> 256, relu  (fp32)
        fin = P[T % 2]
        sf01 = hpool.tile([128, BC], F32, tag="sf01", name="sf01")
        sf23 = hpool.tile([128, BC], F32, tag="sf23", name="sf23")
        s23b = hpool.tile([128, BC], BF16, tag="s23b", name="s23b")
        sf4b = hpool.tile([64, BC], BF16, tag="sf4b", name="sf4b")
        sf4 = hpool.tile([64, BC], F32, tag="sf4", name="sf4")
        nc.vector.tensor_copy(out=sf01[:], in_=fin[2][:])    # [s0; s1]
        # [s2; s3] lives split: s2 = P3[64:128], s3 = P0[0:64]; partition
        # moves go through DMA, then one cast to fp32.
        nc.sync.dma_start(out=s23b[0:64, :], in_=fin[3][64:128, :])
        nc.sync.dma_start(out=s23b[64:128, :], in_=fin[0][0:64, :])
        nc.vector.tensor_copy(out=sf23[:], in_=s23b[:])
        nc.sync.dma_start(out=sf4b[:], in_=fin[0][64:128, :])
        nc.vector.tensor_copy(out=sf4[:], in_=sf4b[:])
        hout = [hpool.tile([128, BC], F32, tag=f"hout{mc}", name=f"hout{mc}")
                for mc in range(2)]
        for mc in range(2):
            for ncs in range(NCS):
                pt = ps1.tile([128, NW], F32, tag="ps1", name="ps1t")
                sl = slice(ncs * NW, (ncs + 1) * NW)
                nc.tensor.matmul(pt[:], lhsT=wo1a[:, mc * 128:(mc + 1) * 128],
                                 rhs=xt[:, sl], start=True, stop=False)
                nc.tensor.matmul(pt[:], lhsT=wo1b[:, mc * 128:(mc + 1) * 128],
                                 rhs=sf01[:, sl], start=False, stop=False)
                nc.tensor.matmul(pt[:], lhsT=wo1c[:, mc * 128:(mc + 1) * 128],
                                 rhs=sf23[:, sl], start=False, stop=False)
                nc.tensor.matmul(pt[:], lhsT=wo1d[:, mc * 128:(mc + 1) * 128],
                                 rhs=sf4[:, sl], start=False, stop=True)
                nc.scalar.activation(hout[mc][:, sl], pt[:], AF.Relu,
                                     bias=bo1[:, mc:mc + 1])
        nc.sync.dma_start(out=h0_d[:], in_=hout[0][:])
        nc.sync.dma_start(out=h1_d[:], in_=hout[1][:])

    nc.compile()
    return nc


_NC = None


def _get_nc():
    global _NC
    if _NC is None:
        _NC = _build_nc()
    return _NC


def _prep_inputs(x, W1, b1, W2, b2, W3, b3):
    """Host-side packing of weights and the parent-gathered x sequence."""
    bf = ml_dtypes.bfloat16
    xs = x.reshape(B, NODE, T)                      # [B, node, t]

    # xg[32g + 2n + j, t, b] = x[b, parents[n][j], t], replicated g=0..3
    xg = np.zeros((XROWS, T, B), dtype=bf)
    blk = np.empty((10, T, B), dtype=bf)
    for n in range(NODE):
        for j in range(2):
            blk[2 * n + j] = xs[:, PARENTS[n][j], :].T.astype(bf)
    for g in range(4):
        xg[32 * g:32 * g + 10] = blk
    xt = np.ascontiguousarray(xs[:, :, T - 1].T.astype(np.float32))  # [5, B]

    w1s = np.empty((128, NODE * F0), dtype=bf)
    # x-injection block weights: rows 32g+2n'+j match xg rows; only node n's
    # rows are nonzero in node n's column block.
    w1x = np.zeros((XROWS, NODE * F0), dtype=bf)
    for n in range(NODE):
        w1s[:, n * F0:(n + 1) * F0] = W1[n, 2:130, :].astype(bf)
        for g in range(4):
            w1x[32 * g + 2 * n:32 * g + 2 * n + 2, n * F0:(n + 1) * F0] = \
                W1[n, 0:2, :].astype(bf)
    # node 4 reads P4 stored as [s3; s2] (pair23's direct tanh layout):
    # swap its state-weight halves to match.
    w1s[0:64, 4 * F0:5 * F0] = W1[4, 66:130, :].astype(bf)
    w1s[64:128, 4 * F0:5 * F0] = W1[4, 2:66, :].astype(bf)
    f8 = ml_dtypes.float8_e4m3
    w2s = np.empty((128, NODE * 2, 2, 128), dtype=f8)
    for n in range(NODE):
        for mc in range(2):
            for kc in range(2):
                w2s[:, n * 2 + mc, kc, :] = \
                    W2[n, kc * 128:(kc + 1) * 128,
                       mc * 128:(mc + 1) * 128].astype(f8)
    w3s = np.empty((128, NODE * 2 * FML), dtype=bf)
    for n in range(NODE):
        for kc in range(2):
            w3s[:, (n * 2 + kc) * FML:(n * 2 + kc + 1) * FML] = \
                W3[n, kc * 128:(kc + 1) * 128, :].astype(bf)
    b1p = np.empty((128, NODE * 2), dtype=np.float32)
    b2p = np.empty((128, NODE * 2), dtype=np.float32)
    # b3 packed: col0 = [b3[0];b3[1]] (pair01), col1 = [b3[3];b3[2]]
    # (pair23, row-swapped), col2 = [junk; b3[4]] (solo4, rows 64:128)
    b3p = np.zeros((128, 3), dtype=np.float32)
    for n in range(NODE):
        for mc in range(2):
            b1p[:, n * 2 + mc] = b1[n, mc * 128:(mc + 1) * 128]
            b2p[:, n * 2 + mc] = b2[n, mc * 128:(mc + 1) * 128]
    b3p[0:64, 0] = b3[0]
    b3p[64:128, 0] = b3[1]
    b3p[0:64, 1] = b3[3]
    b3p[64:128, 1] = b3[2]
    b3p[64:128, 2] = b3[4]
    return xg, xt, w1s, w1x, w2s, w3s, b1p, b2p, b3p


def _make_in_maps(x, W1, b1, W2, b2, W3, b3, Wo1, bo1):
    xg, xt, w1s, w1x, w2s, w3s, b1p, b2p, b3p = _prep_inputs(
        np.asarray(x, dtype=np.float32), np.asarray(W1), np.asarray(b1),
        np.asarray(W2), np.asarray(b2), np.asarray(W3), np.asarray(b3))
    Wo1 = np.asarray(Wo1, dtype=np.float32)
    shared = dict(w1s=w1s, w1x=w1x, w2s=w2s, w3s=w3s, b1p=b1p, b2p=b2p,
                  b3p=b3p,
                  wo1a=np.ascontiguousarray(Wo1[0:5, :]),
                  wo1b=np.ascontiguousarray(Wo1[5:133, :]),
                  wo1c=np.ascontiguousarray(Wo1[133:261, :]),
                  wo1d=np.ascontiguousarray(Wo1[261:325, :]),
                  bo1p=np.asarray(bo1, dtype=np.float32).reshape(2, 128).T.copy())
    in_maps = []
    for c in range(NCORES):
        sl = slice(c * BC, (c + 1) * BC)
        xgc = np.ascontiguousarray(xg[:, :, sl]).reshape(XROWS, T // TC, TC * BC)
        in_maps.append(dict(shared, xg=xgc,
                            xt=np.ascontiguousarray(xt[:, sl])))
    return in_maps


def kernel(x, W1, b1, W2, b2, W3, b3, Wo1, bo1, gamma, beta, Wo2, bo2):
    in_maps = _make_in_maps(x, W1, b1, W2, b2, W3, b3, Wo1, bo1)
    nc = _get_nc()
    res = run_bass_kernel_spmd(nc, in_maps, core_ids=list(range(NCORES)))

    # gather h = relu(feat @ Wo1 + bo1), shape [8192, 256]
    h = np.empty((B, 256), dtype=np.float32)
    for c, r in enumerate(res.results):
        sl = slice(c * BC, (c + 1) * BC)
        h[sl, 0:128] = r["h0"].T
        h[sl, 128:256] = r["h1"].T

    # ---- host tail: BatchNorm (training-mode batch stats) + Wo2 + softmax
    mu = h.mean(axis=0)
    var = ((h - mu) ** 2).mean(axis=0)
    hn = (h - mu) / np.sqrt(var + BN_EPS) * np.asarray(gamma) + np.asarray(beta)
    logits = hn @ np.asarray(Wo2) + np.asarray(bo2)
    e = np.exp(logits - logits.max(axis=1, keepdims=True))
    return (e / e.sum(axis=1, keepdims=True)).astype(np.float32)


# revision 47
# speedup vs baseline: 1.0160x; 1.0160x over previous
"""Trainium2 Bass kernel for the GNN-RNN diagnoser.

Model (per reference): x [8192,1,5,128]; 128 sequential timesteps; each step
runs 5 per-node MLPs (130->256->256->64, relu/relu/tanh) where node inputs are
[x of 2 parents ; states of 2 parents]; then head 325->256 relu, BatchNorm
(training mode, full-batch stats), 256->7, softmax.

Strategy (v5):
  - Data-parallel: batch 8192 -> 8 cores x 1024; feature-major layout
    [features, batch] on device.
  - States kept as 5 "parent pair" SBUF tensors P[m] = [s_p0 ; s_p1] (128 x B)
    in bf16, ping-ponged across steps, so layer-1 is one K=128 matmul per
    m-chunk plus a small K=10 x-injection matmul into the same PSUM bank.
  - L1 emission: 4 full-row state matmuls first (start=True), then the 4
    x-injection matmuls back-to-back on the 4 PE row groups (concurrent).
  - Layer-2 in fp8-e4m3 DoubleRow: h1 written as [128,2(kc),B] fp8, W2
    packed [128,2,128]; one matmul contracts K=256 at 2 MACs/cycle.
    (Simulated end-to-end error 6e-3 vs the 2e-2 budget.)
  - The whole per-node chain is ncs-major (512-batch half-slices flow
    L1->relu1->L2->relu2->L3->tanh independently) to shorten the recurrence
    critical path.
  - Layer-3 column-paired; pair (2,3) swaps rows (s3 on 0:64, s2 on 64:128)
    so tanh writes P0/P3 slots partition-aligned: the recurrence-critical
    state hand-offs never ride a DMA.  P4/P3/P1 secondary slots are filled
    by GpSimd-triggered SBUF DMAs on slack paths.
  - PSUM-draining activations greedily balanced between Vector and Scalar;
    tanh and state hand-offs are scheduler-prioritized.
  - BatchNorm/Wo2/softmax tail on host (needs cross-core batch stats).
"""

import numpy as np
import ml_dtypes
from contextlib import ExitStack

import concourse.bass as bass
import concourse.bacc as bacc
import concourse.tile as tile
from concourse import mybir
from concourse.bass_utils import run_bass_kernel_spmd

F32 = mybir.dt.float32
BF16 = mybir.dt.bfloat16
FP8 = mybir.dt.float8e4
AF = mybir.ActivationFunctionType
ALU = mybir.AluOpType
DR = mybir.MatmulPerfMode.DoubleRow

NCORES = 8
B = 8192
BC = B // NCORES          # 1024 batch per core
T = 128
TC = 8                    # timesteps per xg ring-buffer chunk
NODE = 5
FML = 64
F0 = 256
NCS = 2                   # batch N-chunks of 512 (PSUM bank limit)
NW = BC // NCS            # 512
XROWS = 106               # xg replicated at row groups 0,32,64,96 (10 rows each)

PARENTS = [(3, 4), (0, 4), (0, 1), (1, 2), (2, 3)]
BN_EPS = 1e-5


def _build_nc():
    nc = bacc.Bacc("TRN2", target_bir_lowering=False, debug=False,
                   num_devices=NCORES)

    # ---- DRAM I/O -------------------------------------------------------
    xg_d = nc.dram_tensor("xg", [XROWS, T // TC, TC * BC], BF16, kind="ExternalInput")
    xt_d = nc.dram_tensor("xt", [NODE, BC], F32, kind="ExternalInput")
    w1s_d = nc.dram_tensor("w1s", [128, NODE * F0], BF16, kind="ExternalInput")
    w1x_d = nc.dram_tensor("w1x", [XROWS, NODE * F0], BF16, kind="ExternalInput")
    w2s_d = nc.dram_tensor("w2s", [128, NODE * 2, 2, 128], FP8, kind="ExternalInput")
    w3s_d = nc.dram_tensor("w3s", [128, NODE * 2 * FML], BF16, kind="ExternalInput")
    b1_d = nc.dram_tensor("b1p", [128, NODE * 2], F32, kind="ExternalInput")
    b2_d = nc.dram_tensor("b2p", [128, NODE * 2], F32, kind="ExternalInput")
    b3_d = nc.dram_tensor("b3p", [128, 3], F32, kind="ExternalInput")
    wo1a_d = nc.dram_tensor("wo1a", [NODE, F0], F32, kind="ExternalInput")
    wo1b_d = nc.dram_tensor("wo1b", [128, F0], F32, kind="ExternalInput")
    wo1c_d = nc.dram_tensor("wo1c", [128, F0], F32, kind="ExternalInput")
    wo1d_d = nc.dram_tensor("wo1d", [64, F0], F32, kind="ExternalInput")
    bo1_d = nc.dram_tensor("bo1p", [128, 2], F32, kind="ExternalInput")
    h0_d = nc.dram_tensor("h0", [128, BC], F32, kind="ExternalOutput")
    h1_d = nc.dram_tensor("h1", [128, BC], F32, kind="ExternalOutput")

    with tile.TileContext(nc) as tc, ExitStack() as ctx:
        const = ctx.enter_context(tc.tile_pool(name="const", bufs=1))
        spool = ctx.enter_context(tc.tile_pool(name="state", bufs=1))
        apool = ctx.enter_context(tc.tile_pool(name="act", bufs=3))
        xpool = ctx.enter_context(tc.tile_pool(name="xgr", bufs=4))
        hpool = ctx.enter_context(tc.tile_pool(name="head", bufs=1))
        ps1 = ctx.enter_context(tc.tile_pool(name="ps1", bufs=4, space="PSUM"))
        ps2 = ctx.enter_context(tc.tile_pool(name="ps2", bufs=3, space="PSUM"))
        ps3 = ctx.enter_context(tc.tile_pool(name="ps3", bufs=1, space="PSUM"))

        # Greedy build-time load balancing of PSUM-draining activations
        # between the Vector and Scalar engines (tanh is Scalar-only).
        ebusy = {"v": 0.0, "s": 0.0}

        def relu_ts(out, in_, bias_col, cost_v=700.0, cost_s=640.0,
                    force=None):
            """relu(x + b): tensor_scalar on DVE or activation on ScalarE.

            Returns the engine used; force="v"/"s" overrides the greedy
            choice (used to pin the two halves of a relu1 pair to opposite
            engines so they always run in parallel)."""
            on_v = (ebusy["v"] + cost_v <= ebusy["s"] + cost_s
                    if force is None else force == "v")
            if on_v:
                ebusy["v"] += cost_v
                nc.vector.tensor_scalar(out, in_, bias_col, 0.0,
                                        ALU.add, ALU.max)
                return "v"
            ebusy["s"] += cost_s
            nc.scalar.activation(out, in_, AF.Relu, bias=bias_col)
            return "s"

        def load(pool, dram, shape, dtype, tag, eng=None):
            t = pool.tile(shape, dtype, tag=tag, name=tag)
            (eng or nc.sync).dma_start(out=t[:], in_=dram[:])
            return t

        # big constants split across the sync- and scalar-triggered DMA
        # queues so the startup transfers overlap
        w1s = load(const, w1s_d, [128, NODE * F0], BF16, "w1s")
        w1x = load(const, w1x_d, [XROWS, NODE * F0], BF16, "w1x",
                   eng=nc.scalar)
        w2s = load(const, w2s_d, [128, NODE * 2, 2, 128], FP8, "w2s",
                   eng=nc.scalar)
        w3s = load(const, w3s_d, [128, NODE * 2 * FML], BF16, "w3s")
        b1 = load(const, b1_d, [128, NODE * 2], F32, "b1", eng=nc.scalar)
        b2 = load(const, b2_d, [128, NODE * 2], F32, "b2")
        b3 = load(const, b3_d, [128, 3], F32, "b3")

        # state pair buffers, ping-pong
        P = [[spool.tile([128, BC], BF16, tag=f"P{ph}_{m}", name=f"P{ph}_{m}")
              for m in range(NODE)] for ph in range(2)]
        for m in range(NODE):
            nc.vector.memset(P[0][m][:], 0.0)

        xg_ring = {}

        def layers12(n, cur, xr, tl, hot_l2=False):
            """L1 + L2 for node n; returns the two h2 M-chunk tiles.

            ncs-major: the two 512-batch half-slices flow through
            L1 -> relu1 -> L2(DoubleRow) -> relu2 independently."""
            pts = [ps1.tile([128, NW], F32, tag="ps1", name="ps1t")
                   for _ in range(4)]          # index 2*ncs + mc
            # per half-slice: 2 state matmuls then the 2 x-injection closers
            # (row-tiled, concurrent) so relu1(ncs0) is ready two matmul
            # slots earlier and the ncs0 stream truly leads.
            for ncs in range(NCS):
                for mc in range(2):
                    nc.tensor.matmul(
                        pts[2 * ncs + mc][:],
                        lhsT=w1s[:, n * F0 + mc * 128: n * F0 + (mc + 1) * 128],
                        rhs=cur[n][:, ncs * NW:(ncs + 1) * NW],
                        start=True, stop=False)
                for mc in range(2):
                    g = 2 * ncs + mc
                    rb = 32 * g
                    nc.tensor.matmul(
                        pts[g][:],
                        lhsT=w1x[rb:rb + 10, n * F0 + mc * 128:n * F0 + (mc + 1) * 128],
                        rhs=xr[rb:rb + 10, tl, ncs * NW:(ncs + 1) * NW],
                        start=False, stop=True, tile_position=(rb, 0))
            h1dr = apool.tile([128, 2, BC], FP8, tag="h1dr", name="h1dr")
            h2t = [apool.tile([128, BC], BF16, tag=f"h2_{mc}", name=f"h2_{mc}",
                              bufs=5)
                   for mc in range(2)]
            for ncs in range(NCS):
                sl = slice(ncs * NW, (ncs + 1) * NW)
                with tc.high_priority(offset=90):
                    e0 = relu_ts(h1dr[:, 0, sl], pts[2 * ncs + 0][:],
                                 b1[:, n * 2 + 0: n * 2 + 1])
                    relu_ts(h1dr[:, 1, sl], pts[2 * ncs + 1][:],
                            b1[:, n * 2 + 1: n * 2 + 2],
                            force=("s" if e0 == "v" else "v"))
                pt2 = [ps2.tile([128, NW], F32, tag="ps2", name="ps2t")
                       for _ in range(2)]
                # one DoubleRow matmul contracts all K=256 per bank
                for mc in range(2):
                    nc.tensor.matmul(
                        pt2[mc][:],
                        lhsT=w2s[:, n * 2 + mc, :, :],
                        rhs=h1dr[:, :, sl],
                        start=True, stop=True, perf_mode=DR)
                if hot_l2:
                    with tc.high_priority(offset=90):
                        for mc in range(2):
                            relu_ts(h2t[mc][:, sl], pt2[mc][:],
                                    b2[:, n * 2 + mc: n * 2 + mc + 1])
                else:
                    for mc in range(2):
                        relu_ts(h2t[mc][:, sl], pt2[mc][:],
                                b2[:, n * 2 + mc: n * 2 + mc + 1])
            return h2t

        def l3_pair01(h2A, h2B, nxt):
            """Column-paired DoubleRow layer 3 for nodes (0,1): psum rows
            = [s0; s1].

            tanh writes P2 (= [s0;s1]) directly; s0 -> P1[0:64] same-row
            copy (GpSimd); s1 -> P3[0:64] is cross-partition, DMA'd on a
            slack path (node 3 runs last next step)."""
            dest = nxt[2]
            for ncs in range(NCS):
                sl = slice(ncs * NW, (ncs + 1) * NW)
                pt = ps3.tile([128, NW], F32, tag="ps3", name="ps3t")
                for kc in range(2):
                    nc.tensor.matmul(
                        pt[0:64, :],
                        lhsT=w3s[:, (0 * 2 + kc) * FML:(0 * 2 + kc + 1) * FML],
                        rhs=h2A[kc][:, sl],
                        start=(kc == 0), stop=(kc == 1), tile_position=(0, 0))
                    nc.tensor.matmul(
                        pt[64:128, :],
                        lhsT=w3s[:, (1 * 2 + kc) * FML:(1 * 2 + kc + 1) * FML],
                        rhs=h2B[kc][:, sl],
                        start=(kc == 0), stop=(kc == 1), tile_position=(0, 64))
                ebusy["s"] += 613.0
                with tc.high_priority(offset=130):
                    nc.scalar.activation(dest[:, sl], pt[:], AF.Tanh,
                                         bias=b3[:, 0:1])
            with tc.high_priority(offset=130):
                ebusy["v"] += 420.0
                nc.vector.tensor_copy(out=nxt[1][0:64, :], in_=dest[0:64, :])
                nc.gpsimd.dma_start(out=nxt[3][0:64, :], in_=dest[64:128, :])

        def l3_pair23(h2_3, h2_2, nxt):
            """Column-paired DoubleRow layer 3 for nodes (3,2): psum rows
            = [s3; s2].

            Node 4's W1 state-weight halves are swapped host-side, so P4
            stores [s3; s2] and tanh writes it DIRECTLY.  The other
            consumers are partition-aligned copies: s3 -> P0[0:64] (node 0,
            Vector, urgent), s2 -> P3[64:128] (node 3, GpSimd, slack)."""
            dest = nxt[4]
            for ncs in range(NCS):
                sl = slice(ncs * NW, (ncs + 1) * NW)
                pt = ps3.tile([128, NW], F32, tag="ps3", name="ps3t")
                for kc in range(2):
                    nc.tensor.matmul(
                        pt[0:64, :],
                        lhsT=w3s[:, (3 * 2 + kc) * FML:(3 * 2 + kc + 1) * FML],
                        rhs=h2_3[kc][:, sl],
                        start=(kc == 0), stop=(kc == 1), tile_position=(0, 0))
                    nc.tensor.matmul(
                        pt[64:128, :],
                        lhsT=w3s[:, (2 * 2 + kc) * FML:(2 * 2 + kc + 1) * FML],
                        rhs=h2_2[kc][:, sl],
                        start=(kc == 0), stop=(kc == 1), tile_position=(0, 64))
                ebusy["s"] += 613.0
                with tc.high_priority(offset=130):
                    nc.scalar.activation(dest[:, sl], pt[:], AF.Tanh,
                                         bias=b3[:, 1:2])
            with tc.high_priority(offset=130):
                ebusy["v"] += 840.0
                nc.vector.tensor_copy(out=nxt[0][0:64, :], in_=dest[0:64, :])
                nc.vector.tensor_copy(out=nxt[3][64:128, :], in_=dest[64:128, :])

        def l3_solo4(h2t, nxt):
            """Layer 3 for node 4: both its slots sit at row 64."""
            for ncs in range(NCS):
                sl = slice(ncs * NW, (ncs + 1) * NW)
                pt = ps3.tile([128, NW], F32, tag="ps3", name="ps3t")
                for kc in range(2):
                    nc.tensor.matmul(
                        pt[64:128, :],
                        lhsT=w3s[:, (4 * 2 + kc) * FML:(4 * 2 + kc + 1) * FML],
                        rhs=h2t[kc][:, sl],
                        start=(kc == 0), stop=(kc == 1), tile_position=(0, 64))
                ebusy["s"] += 613.0
                with tc.high_priority(offset=130):
                    nc.scalar.activation(nxt[0][64:128, sl], pt[64:128, :],
                                         AF.Tanh, bias=b3[64:128, 2:3])
            with tc.high_priority(offset=130):
                ebusy["v"] += 420.0
                nc.vector.tensor_copy(out=nxt[1][64:128, :], in_=nxt[0][64:128, :])

        def fetch_chunk(c):
            if c in xg_ring or c >= T // TC:
                return
            # issue on GpSimd: its instruction stream is otherwise empty, so
            # the trigger fires as soon as the ring slot frees (true prefetch)
            xr = xpool.tile([XROWS, TC, BC], BF16, tag="xgr", name="xgr")
            q = (TC // 4) * BC
            for s4 in range(4):
                nc.gpsimd.dma_start(
                    out=xr[:, s4 * (TC // 4):(s4 + 1) * (TC // 4), :],
                    in_=xg_d[:, c, s4 * q:(s4 + 1) * q])
            xg_ring[c] = xr

        def step(t_abs):
            cur = P[t_abs % 2]
            nxt = P[(t_abs + 1) % 2]
            c, tl = divmod(t_abs, TC)
            fetch_chunk(c)
            if tl == 2:
                fetch_chunk(c + 1)   # prefetch next chunk early in this one
            xr = xg_ring[c]
            # Node order [2,0,1,4,3]: consumers of the earliest-completed state
            # pairs run first next step, so cross-step producer->consumer
            # latency (tanh + copy/DMA chains) is hidden.
            h2_2 = layers12(2, cur, xr, tl, hot_l2=True)
            h2_1 = layers12(1, cur, xr, tl, hot_l2=True)
            h2_0 = layers12(0, cur, xr, tl, hot_l2=True)
            l3_pair01(h2_0, h2_1, nxt)
            h2_4 = layers12(4, cur, xr, tl, hot_l2=True)
            l3_solo4(h2_4, nxt)
            h2_3 = layers12(3, cur, xr, tl, hot_l2=True)
            l3_pair23(h2_3, h2_2, nxt)

        for t_abs in range(T):
            step(t_abs)

        # head-only constants: loaded during the loop, not at startup
        xt = load(const, xt_d, [NODE, BC], F32, "xt")
        wo1a = load(const, wo1a_d, [NODE, F0], F32, "wo1a")
        wo1b = load(const, wo1b_d, [128, F0], F32, "wo1b")
        wo1c = load(const, wo1c_d, [128, F0], F32, "wo1c")
        wo1d = load(const, wo1d_d, [64, F0], F32, "wo1d")
        bo1 = load(const, bo1_d, [128, 2], F32, "bo1")

        # ---- head: feat = [x_T(5); s0..s4(320)] -> 256, relu  (fp32)
        fin = P[T % 2]
        sf01 = hpool.tile([128, BC], F32, tag="sf01", name="sf01")
        sf23 = hpool.tile([128, BC], F32, tag="sf23", name="sf23")
        s23b = hpool.tile([128, BC], BF16, tag="s23b", name="s23b")
        sf4b = hpool.tile([64, BC], BF16, tag="sf4b", name="sf4b")
        sf4 = hpool.tile([64, BC], F32, tag="sf4", name="sf4")
        nc.vector.tensor_copy(out=sf01[:], in_=fin[2][:])    # [s0; s1]
        # [s2; s3] lives split: s2 = P3[64:128], s3 = P0[0:64]; partition
        # moves go through DMA, then one cast to fp32.
        nc.sync.dma_start(out=s23b[0:64, :], in_=fin[3][64:128, :])
        nc.sync.dma_start(out=s23b[64:128, :], in_=fin[0][0:64, :])
        nc.vector.tensor_copy(out=sf23[:], in_=s23b[:])
        nc.sync.dma_start(out=sf4b[:], in_=fin[0][64:128, :])
        nc.vector.tensor_copy(out=sf4[:], in_=sf4b[:])
        hout = [hpool.tile([128, BC], F32, tag=f"hout{mc}", name=f"hout{mc}")
                for mc in range(2)]
        for mc in range(2):
            for ncs in range(NCS):
                pt = ps1.tile([128, NW], F32, tag="ps1", name="ps1t")
                sl = slice(ncs * NW, (ncs + 1) * NW)
                nc.tensor.matmul(pt[:], lhsT=wo1a[:, mc * 128:(mc + 1) * 128],
                                 rhs=xt[:, sl], start=True, stop=False)
                nc.tensor.matmul(pt[:], lhsT=wo1b[:, mc * 128:(mc + 1) * 128],
                                 rhs=sf01[:, sl], start=False, stop=False)
                nc.tensor.matmul(pt[:], lhsT=wo1c[:, mc * 128:(mc + 1) * 128],
                                 rhs=sf23[:, sl], start=False, stop=False)
                nc.tensor.matmul(pt[:], lhsT=wo1d[:, mc * 128:(mc + 1) * 128],
                                 rhs=sf4[:, sl], start=False, stop=True)
                nc.scalar.activation(hout[mc][:, sl], pt[:], AF.Relu,
                                     bias=bo1[:, mc:mc + 1])
        nc.sync.dma_start(out=h0_d[:], in_=hout[0][:])
        nc.sync.dma_start(out=h1_d[:], in_=hout[1][:])

    nc.compile()
    return nc


_NC = None


def _get_nc():
    global _NC
    if _NC is None:
        _NC = _build_nc()
    return _NC


def _prep_inputs(x, W1, b1, W2, b2, W3, b3):
    """Host-side packing of weights and the parent-gathered x sequence."""
    bf = ml_dtypes.bfloat16
    xs = x.reshape(B, NODE, T)                      # [B, node, t]

    # xg[32g + 2n + j, t, b] = x[b, parents[n][j], t], replicated g=0..3
    xg = np.zeros((XROWS, T, B), dtype=bf)
    blk = np.empty((10, T, B), dtype=bf)
    for n in range(NODE):
        for j in range(2):
            blk[2 * n + j] = xs[:, PARENTS[n][j], :].T.astype(bf)
    for g in range(4):
        xg[32 * g:32 * g + 10] = blk
    xt = np.ascontiguousarray(xs[:, :, T - 1].T.astype(np.float32))  # [5, B]

    w1s = np.empty((128, NODE * F0), dtype=bf)
    # x-injection block weights: rows 32g+2n'+j match xg rows; only node n's
    # rows are nonzero in node n's column block.
    w1x = np.zeros((XROWS, NODE * F0), dtype=bf)
    for n in range(NODE):
        w1s[:, n * F0:(n + 1) * F0] = W1[n, 2:130, :].astype(bf)
        for g in range(4):
            w1x[32 * g + 2 * n:32 * g + 2 * n + 2, n * F0:(n + 1) * F0] = \
                W1[n, 0:2, :].astype(bf)
    # node 4 reads P4 stored as [s3; s2] (pair23's direct tanh layout):
    # swap its state-weight halves to match.
    w1s[0:64, 4 * F0:5 * F0] = W1[4, 66:130, :].astype(bf)
    w1s[64:128, 4 * F0:5 * F0] = W1[4, 2:66, :].astype(bf)
    f8 = ml_dtypes.float8_e4m3
    w2s = np.empty((128, NODE * 2, 2, 128), dtype=f8)
    for n in range(NODE):
        for mc in range(2):
            for kc in range(2):
                w2s[:, n * 2 + mc, kc, :] = \
                    W2[n, kc * 128:(kc + 1) * 128,
                       mc * 128:(mc + 1) * 128].astype(f8)
    w3s = np.empty((128, NODE * 2 * FML), dtype=bf)
    for n in range(NODE):
        for kc in range(2):
            w3s[:, (n * 2 + kc) * FML:(n * 2 + kc + 1) * FML] = \
                W3[n, kc * 128:(kc + 1) * 128, :].astype(bf)
    b1p = np.empty((128, NODE * 2), dtype=np.float32)
    b2p = np.empty((128, NODE * 2), dtype=np.float32)
    # b3 packed: col0 = [b3[0];b3[1]] (pair01), col1 = [b3[3];b3[2]]
    # (pair23, row-swapped), col2 = [junk; b3[4]] (solo4, rows 64:128)
    b3p = np.zeros((128, 3), dtype=np.float32)
    for n in range(NODE):
        for mc in range(2):
            b1p[:, n * 2 + mc] = b1[n, mc * 128:(mc + 1) * 128]
            b2p[:, n * 2 + mc] = b2[n, mc * 128:(mc + 1) * 128]
    b3p[0:64, 0] = b3[0]
    b3p[64:128, 0] = b3[1]
    b3p[0:64, 1] = b3[3]
    b3p[64:128, 1] = b3[2]
    b3p[64:128, 2] = b3[4]
    return xg, xt, w1s, w1x, w2s, w3s, b1p, b2p, b3p


def _make_in_maps(x, W1, b1, W2, b2, W3, b3, Wo1, bo1):
    xg, xt, w1s, w1x, w2s, w3s, b1p, b2p, b3p = _prep_inputs(
        np.asarray(x, dtype=np.float32), np.asarray(W1), np.asarray(b1),
        np.asarray(W2), np.asarray(b2), np.asarray(W3), np.asarray(b3))
    Wo1 = np.asarray(Wo1, dtype=np.float32)
    shared = dict(w1s=w1s, w1x=w1x, w2s=w2s, w3s=w3s, b1p=b1p, b2p=b2p,
                  b3p=b3p,
                  wo1a=np.ascontiguousarray(Wo1[0:5, :]),
                  wo1b=np.ascontiguousarray(Wo1[5:133, :]),
                  wo1c=np.ascontiguousarray(Wo1[133:261, :]),
                  wo1d=np.ascontiguousarray(Wo1[261:325, :]),
                  bo1p=np.asarray(bo1, dtype=np.float32).reshape(2, 128).T.copy())
    in_maps = []
    for c in range(NCORES):
        sl = slice(c * BC, (c + 1) * BC)
        xgc = np.ascontiguousarray(xg[:, :, sl]).reshape(XROWS, T // TC, TC * BC)
        in_maps.append(dict(shared, xg=xgc,
                            xt=np.ascontiguousarray(xt[:, sl])))
    return in_maps


def kernel(x, W1, b1, W2, b2, W3, b3, Wo1, bo1, gamma, beta, Wo2, bo2):
    in_maps = _make_in_maps(x, W1, b1, W2, b2, W3, b3, Wo1, bo1)
    nc = _get_nc()
    res = run_bass_kernel_spmd(nc, in_maps, core_ids=list(range(NCORES)))

    # gather h = relu(feat @ Wo1 + bo1), shape [8192, 256]
    h = np.empty((B, 256), dtype=np.float32)
    for c, r in enumerate(res.results):
        sl = slice(c * BC, (c + 1) * BC)
        h[sl, 0:128] = r["h0"].T
        h[sl, 128:256] = r["h1"].T

    # ---- host tail: BatchNorm (training-mode batch stats) + Wo2 + softmax
    mu = h.mean(axis=0)
    var = ((h - mu) ** 2).mean(axis=0)
    hn = (h - mu) / np.sqrt(var + BN_EPS) * np.asarray(gamma) + np.asarray(beta)
    logits = hn @ np.asarray(Wo2) + np.asarray(bo2)
    e = np.exp(logits - logits.max(axis=1, keepdims=True))
    return (e / e.sum(axis=1, keepdims=True)).astype(np.float32)
